# revision 1
# baseline (speedup 1.0000x reference)
"""LocalMHA (windowed attention, window=128, look_backward=1, RoPE) on 8 TRN2 cores.

Sharding: sequence-parallel, no collectives. Core c handles batch c//2,
sequence half c%2 (2048 query tokens + a 128-token look-backward halo whose
x rows ride along in the core's input shard; zeros at a true sequence start,
where the mask kills the backward keys anyway).

Layout trick: within each 128-row (2-head) block of the head-transposed q/k,
rows are permuted to [hA_d0-31 | hB_d0-31 | hA_d32-63 | hB_d32-63] (host-side
column permutation of W_qkv). The rotate_half partner is then r^64, so RoPE
needs only full-width partition-shifted multiplies (no 32-row fragments), with
the sin sign folded host-side. Scores contract each head's d over two 32-row
chunks (PSUM accumulation) — the dot product is invariant to the d-permutation.

Dtypes: projections and scores in fp32r (full PE rate at free>=256, ~1.6e-4);
attention probabilities and v in bf16 (free-dim-128 matmuls at full rate).

Engine split: PE matmuls/transposes; DVE elementwise (RoPE, mask-add,
normalize, most evictions); ACT exp(+fused row-sum) and the remaining psum
evictions. GPSIMD is left idle on purpose — it shares its SBUF port with DVE
under an exclusive lock, so "offloading" to it stalls DVE (measured +35%).
"""

import numpy as np
from contextlib import ExitStack

import concourse.bacc as bacc
import concourse.tile as tile
import concourse.mybir as mybir
from concourse.bass_utils import run_bass_kernel_spmd
from concourse.masks import make_identity

# Problem shape (hardcoded per contract)
B, N, D = 4, 4096, 1024
H, DH, WS = 16, 64, 128
THETA = 10000.0
N3 = 3 * H * DH            # 3072
NCORES = 8
HALF = N // 2              # 2048 query tokens per core
NT = HALF + WS             # 2176 tokens incl halo window
NWIN = HALF // WS          # 16 query windows
SCALE = DH ** -0.5
NEG = -1.0e9

F32 = mybir.dt.float32
F32R = mybir.dt.float32r
BF16 = mybir.dt.bfloat16
ADD = mybir.AluOpType.add
MUL = mybir.AluOpType.mult
EXP = mybir.ActivationFunctionType.Exp

# token chunks for phases A/B (start, len); 128-aligned, len<=512
CHUNKS = [(0, 512), (512, 512), (1024, 512), (1536, 512), (2048, 128)]


def _rope(nc, tmpp, src_psum, dst, L, rp, ci, si):
    """dst[:, :L] = src*cos + rot64(src)*sin_signed, straight from PSUM.

    Permuted layout: rotate partner of row r is r^64. The sin tile is indexed
    by SOURCE row with the destination's sign folded in host-side, so each
    multiply's two inputs share a base partition (only outputs are shifted —
    the ISA allows that). All ops stay on DVE: GPSIMD shares its SBUF port
    with DVE under an exclusive lock, so offloading there is a net loss.
    """
    t1 = tmpp.tile([128, 512], F32, tag="t1")
    nc.vector.tensor_tensor(t1[:, :L], src_psum[:, :L], rp[:, ci, :L], MUL)
    t2 = tmpp.tile([128, 512], F32, tag="t2")
    nc.vector.tensor_tensor(t2[0:64, :L], src_psum[64:128, :L],
                            rp[64:128, si, :L], MUL)
    nc.vector.tensor_tensor(t2[64:128, :L], src_psum[0:64, :L],
                            rp[0:64, si, :L], MUL)
    nc.vector.tensor_tensor(dst[:, :L], t1[:, :L], t2[:, :L], ADD)


def _build(reps=1):
    nc = bacc.Bacc("TRN2", target_bir_lowering=False, debug=False,
                   enable_asserts=False, num_devices=NCORES)

    xs = nc.dram_tensor("xs", [NT, D], F32R, kind="ExternalInput").ap()
    wqkv = nc.dram_tensor("wqkv", [D, N3], F32R, kind="ExternalInput").ap()
    wout = nc.dram_tensor("wout", [D, D], F32R, kind="ExternalInput").ap()
    # 0:qcos 1:qsin 2:kcos_cur 3:ksin_cur 4:kcos_prev 5:ksin_prev
    ropes = nc.dram_tensor("ropes", [6, 128, 512], F32, kind="ExternalInput").ap()
    masks = nc.dram_tensor("masks", [2, 128, 256], F32, kind="ExternalInput").ap()
    out = nc.dram_tensor("out", [HALF, D], F32, kind="ExternalOutput").ap()

    # internal DRAM staging
    qrope = nc.dram_tensor("qrope", [D, HALF], F32R).ap()
    k2 = nc.dram_tensor("k2", [D, NWIN, 2 * WS], F32R).ap()
    vstage = nc.dram_tensor("vstage", [NT, D], BF16).ap()

    with tile.TileContext(nc) as tc:
        with ExitStack() as top:
            constp = top.enter_context(tc.tile_pool(name="const", bufs=1))
            identf = constp.tile([128, 128], F32, tag="idf")
            make_identity(nc, identf[:])
            identb = constp.tile([128, 128], BF16, tag="idb")
            nc.vector.tensor_copy(identb[:], identf[:])
            identr = constp.tile([128, 128], F32R, tag="idr")
            nc.vector.tensor_copy(identr[:], identf[:])
            rp = constp.tile([128, 6, 512], F32, tag="ropes")
            nc.sync.dma_start(rp[:], ropes.rearrange("r p m -> p r m"))
            mk = constp.tile([128, 2, 256], F32, tag="masks")
            nc.sync.dma_start(mk[:], masks.rearrange("r p m -> p r m"))

            rep_ctx = tc.For_i(0, reps, 1) if reps > 1 else ExitStack()
            top.enter_context(rep_ctx)

            # ---------------- Phase A+B: transpose + QKV + RoPE ----------------
            with ExitStack() as ab:
                wp = ab.enter_context(tc.tile_pool(name="wq", bufs=1))
                w_sb = wp.tile([128, 8, N3], F32R, tag="w")
                nc.sync.dma_start(w_sb[:],
                                  wqkv.rearrange("(c p) n -> p c n", p=128))
                xp = ab.enter_context(tc.tile_pool(name="xst", bufs=2))
                xtp = ab.enter_context(tc.tile_pool(name="xT", bufs=2))
                tmpp = ab.enter_context(tc.tile_pool(name="tmp", bufs=3))
                rop = ab.enter_context(tc.tile_pool(name="ro", bufs=4))
                vp = ab.enter_context(tc.tile_pool(name="vsb", bufs=3))
                tps = ab.enter_context(tc.tile_pool(name="tps", bufs=3, space="PSUM"))
                mps = ab.enter_context(tc.tile_pool(name="mps", bufs=5, space="PSUM"))

                for (s, L) in CHUNKS:
                    nmt = L // 128
                    xT = xtp.tile([128, 8, 512], F32R, tag="xT")
                    for mt in range(nmt):
                        xst = xp.tile([128, D], F32R, tag="x")
                        nc.sync.dma_start(xst[:], xs[s + mt * 128: s + (mt + 1) * 128, :])
                        for kc in range(8):
                            tp = tps.tile([128, 128], F32R, tag="tp")
                            nc.tensor.transpose(tp[:], xst[:, kc * 128:(kc + 1) * 128],
                                                identr[:])
                            nc.scalar.copy(xT[:, kc, mt * 128:(mt + 1) * 128], tp[:])

                    # q^T (+rope) for query tokens of this chunk
                    qs = 128 if s == 0 else 0
                    qL = L - qs
                    if qL > 0:
                        for nch in range(8):
                            qp = mps.tile([128, 512], F32, tag="mm")
                            for kc in range(8):
                                nc.tensor.matmul(qp[:, :qL],
                                                 w_sb[:, kc, nch * 128:(nch + 1) * 128],
                                                 xT[:, kc, qs:qs + qL],
                                                 start=(kc == 0), stop=(kc == 7))
                            qf = rop.tile([128, 512], F32R, tag="ro")
                            _rope(nc, tmpp, qp, qf, qL, rp, 0, 1)
                            q0 = s + qs - 128
                            nc.sync.dma_start(
                                qrope[nch * 128:(nch + 1) * 128, q0:q0 + qL],
                                qf[:, :qL])

                    # k^T with both rope phases
                    for nch in range(8):
                        kp = mps.tile([128, 512], F32, tag="mm")
                        for kc in range(8):
                            nc.tensor.matmul(kp[:, :L],
                                             w_sb[:, kc, 1024 + nch * 128: 1024 + (nch + 1) * 128],
                                             xT[:, kc, 0:L],
                                             start=(kc == 0), stop=(kc == 7))
                        cs = 128 if s == 0 else 0     # halo window has no cur slot
                        if L - cs > 0:
                            kcur = rop.tile([128, 512], F32R, tag="ro")
                            _rope(nc, tmpp, kp, kcur, L, rp, 2, 3)
                            w0 = (s + cs) // 128 - 1
                            nw = (L - cs) // 128
                            nc.sync.dma_start(
                                k2[nch * 128:(nch + 1) * 128, w0:w0 + nw, 128:256],
                                kcur[:, cs:L].rearrange("p (w i) -> p w i", i=128))
                        if s + L <= HALF:             # last window has no next
                            kprv = rop.tile([128, 512], F32R, tag="ro")
                            _rope(nc, tmpp, kp, kprv, L, rp, 4, 5)
                            w0 = s // 128
                            nw = L // 128
                            nc.sync.dma_start(
                                k2[nch * 128:(nch + 1) * 128, w0:w0 + nw, 0:128],
                                kprv[:, 0:L].rearrange("p (w i) -> p w i", i=128))

                    # v in natural layout, bf16
                    for mt in range(nmt):
                        vsb = vp.tile([128, D], BF16, tag="v")
                        for nh in range(2):
                            vq = mps.tile([128, 512], F32, tag="mm")
                            for kc in range(8):
                                nc.tensor.matmul(vq[:],
                                                 xT[:, kc, mt * 128:(mt + 1) * 128],
                                                 w_sb[:, kc, 2048 + nh * 512: 2048 + (nh + 1) * 512],
                                                 start=(kc == 0), stop=(kc == 7))
                            nc.vector.tensor_copy(vsb[:, nh * 512:(nh + 1) * 512], vq[:])
                        nc.sync.dma_start(vstage[s + mt * 128: s + (mt + 1) * 128, :], vsb[:])

            # ---------------- Phase C: windowed attention ----------------
            with ExitStack() as cd:
                atp = cd.enter_context(tc.tile_pool(name="aT", bufs=1))
                aT = atp.tile([128, 8, HALF], F32R, tag="aT")
                with ExitStack() as cc:
                    qwp = cc.enter_context(tc.tile_pool(name="qw", bufs=3))
                    k2p = cc.enter_context(tc.tile_pool(name="k2w", bufs=3))
                    vwp = cc.enter_context(tc.tile_pool(name="vw", bufs=4))
                    ep = cc.enter_context(tc.tile_pool(name="e", bufs=4))
                    pp = cc.enter_context(tc.tile_pool(name="p", bufs=3))
                    ptp = cc.enter_context(tc.tile_pool(name="pt", bufs=3))
                    sump = cc.enter_context(tc.tile_pool(name="sums", bufs=4))
                    sps = cc.enter_context(tc.tile_pool(name="sps", bufs=3, space="PSUM"))
                    tps2 = cc.enter_context(tc.tile_pool(name="tps2", bufs=2, space="PSUM"))
                    aps = cc.enter_context(tc.tile_pool(name="aps", bufs=3, space="PSUM"))

                    # un-permute staged q/k on load: target row groups
                    # [hA_lo, hA_hi, hB_lo, hB_hi] <- permuted-source offsets
                    UNP = (0, 64, 32, 96)
                    vtiles = {}
                    for w in range(NWIN):
                        qsrc = qrope[:, w * 128:(w + 1) * 128] \
                            .rearrange("(c p) m -> p c m", p=128)
                        qw = qwp.tile([128, 8, 128], F32R, tag="qw")
                        for g, off in enumerate(UNP):
                            nc.sync.dma_start(qw[g * 32:(g + 1) * 32, :, :],
                                              qsrc[off:off + 32])
                        ksrc = k2[:, w, :].rearrange("(c p) j -> p c j", p=128)
                        k2w = k2p.tile([128, 8, 256], F32R, tag="k2w")
                        for g, off in enumerate(UNP):
                            nc.sync.dma_start(k2w[g * 32:(g + 1) * 32, :, :],
                                              ksrc[off:off + 32])
                        for vt in ([w, w + 1] if w == 0 else [w + 1]):
                            v_t = vwp.tile([128, D], BF16, tag="vw")
                            nc.sync.dma_start(v_t[:], vstage[vt * 128:(vt + 1) * 128, :])
                            vtiles[vt] = v_t
                        mslot = 0 if w == 0 else 1
                        for h in range(H):
                            blk, sub = h // 2, h % 2
                            po = sub * 64
                            sp = sps.tile([128, 256], F32, tag="s")
                            nc.tensor.matmul(sp[:], qw[po:po + 64, blk, :],
                                             k2w[po:po + 64, blk, :],
                                             start=True, stop=True)
                            em = ep.tile([128, 256], F32, tag="em")
                            nc.vector.tensor_tensor(em[:], sp[:], mk[:, mslot, :], ADD)
                            ee = ep.tile([128, 256], F32, tag="ee")
                            ssum = sump.tile([128, 1], F32, tag="ss")
                            nc.scalar.activation(ee[:], em[:], EXP, accum_out=ssum[:])
                            rr = sump.tile([128, 1], F32, tag="rr")
                            nc.vector.reciprocal(rr[:], ssum[:])
                            pf = pp.tile([128, 256], BF16, tag="pf")
                            nc.vector.tensor_scalar_mul(pf[:], ee[:], rr[:])
                            ptq = tps2.tile([128, 256], BF16, tag="ptq")
                            nc.tensor.transpose(ptq[:, 0:128], pf[:, 0:128], identb[:])
                            nc.tensor.transpose(ptq[:, 128:256], pf[:, 128:256], identb[:])
                            pt = ptp.tile([128, 256], BF16, tag="pt")
                            nc.scalar.copy(pt[:], ptq[:])
                            ap_ = aps.tile([64, 128], F32, tag="ap")
                            nc.tensor.matmul(ap_[:], vtiles[w][:, h * 64:(h + 1) * 64],
                                             pt[:, 0:128], start=True, stop=False)
                            nc.tensor.matmul(ap_[:], vtiles[w + 1][:, h * 64:(h + 1) * 64],
                                             pt[:, 128:256], start=False, stop=True)
                            nc.scalar.copy(aT[(sub) * 64:(sub) * 64 + 64, blk,
                                              w * 128:(w + 1) * 128], ap_[:])
                        vtiles.pop(w - 1, None)

                # ---------------- Phase D: output projection ----------------
                with ExitStack() as dd:
                    wop = dd.enter_context(tc.tile_pool(name="wo", bufs=1))
                    wo = wop.tile([128, 8, D], F32R, tag="wo")
                    nc.sync.dma_start(wo[:], wout.rearrange("(c p) n -> p c n", p=128))
                    outp = dd.enter_context(tc.tile_pool(name="outsb", bufs=3))
                    ops = dd.enter_context(tc.tile_pool(name="ops", bufs=4, space="PSUM"))
                    for mt in range(16):
                        osb = outp.tile([128, D], F32, tag="o")
                        for nh in range(2):
                            op_ = ops.tile([128, 512], F32, tag="op")
                            for kc in range(8):
                                nc.tensor.matmul(op_[:],
                                                 aT[:, kc, mt * 128:(mt + 1) * 128],
                                                 wo[:, kc, nh * 512:(nh + 1) * 512],
                                                 start=(kc == 0), stop=(kc == 7))
                            nc.vector.tensor_copy(osb[:, nh * 512:(nh + 1) * 512], op_[:])
                        nc.sync.dma_start(out[mt * 128:(mt + 1) * 128, :], osb[:])

    nc.compile()
    return nc


_NC = {}


def _get_nc(reps=1):
    if reps not in _NC:
        _NC[reps] = _build(reps)
    return _NC[reps]


# permutation within each 128-row (2-head) block of head-transposed q/k:
# new row r holds old row ((r//32)%2)*64 + (r%32) + 32*(r//64)
_r = np.arange(128)
_PERM = ((_r // 32) % 2) * 64 + (_r % 32) + 32 * (_r // 64)


def _host_inputs(x, W_qkv, W_out):
    # permute q and k column blocks of W_qkv
    W = np.ascontiguousarray(W_qkv, np.float32).copy()
    for sec in range(2):                     # q section, k section
        for b in range(8):
            base = sec * 1024 + b * 128
            W[:, base:base + 128] = W[:, base + _PERM]

    invf = THETA ** (-(np.arange(0, 64, 2) / 64.0))          # [32]
    rows_f = invf[_r % 32]                                   # [128] freq per row
    # sin tiles are indexed by SOURCE row of the rotate (partner r^64);
    # the destination sign is +1 when the source is a hi-half (r>=64).
    rows_s = np.where(_r < 64, 1.0, -1.0)
    mcol = np.arange(512) % 128
    angC = rows_f[:, None] * (128 + mcol)[None, :]
    angP = rows_f[:, None] * mcol[None, :]
    ropes = np.stack([
        SCALE * np.cos(angC),
        SCALE * (rows_s[:, None] * np.sin(angC)),
        np.cos(angC),
        rows_s[:, None] * np.sin(angC),
        np.cos(angP),
        rows_s[:, None] * np.sin(angP),
    ]).astype(np.float32)

    i = np.arange(128)[:, None]
    jj = np.arange(256)[None, :]
    band = (jj >= i) & (jj <= i + 128)
    maskB = np.where(band, 0.0, NEG).astype(np.float32)
    maskA0 = np.where(band & (jj >= 128), 0.0, NEG).astype(np.float32)

    in_maps = []
    for c in range(NCORES):
        bi, hi = c // 2, c % 2
        xsh = np.empty((NT, D), np.float32)
        if hi == 0:
            xsh[:WS] = 0.0
            xsh[WS:] = x[bi, 0:HALF]
            mA = maskA0
        else:
            xsh[:] = x[bi, HALF - WS: N]
            mA = maskB
        in_maps.append({
            "xs": xsh,
            "wqkv": W,
            "wout": np.ascontiguousarray(W_out, np.float32),
            "ropes": ropes,
            "masks": np.stack([mA, maskB]),
        })
    return in_maps


def kernel(x, W_qkv, W_out):
    x = np.asarray(x, np.float32)
    nc = _get_nc()
    in_maps = _host_inputs(x, W_qkv, W_out)
    res = run_bass_kernel_spmd(nc, in_maps, list(range(NCORES)))
    outf = np.empty((B, N, D), np.float32)
    for c in range(NCORES):
        bi, hi = c // 2, c % 2
        outf[bi, hi * HALF:(hi + 1) * HALF] = res.results[c]["out"]
    return outf



# revision 10
# speedup vs baseline: 1.6276x; 1.6276x over previous
"""LocalMHA (windowed attention, window=128, look_backward=1, RoPE) on 8 TRN2 cores.

Sharding: sequence-parallel, no collectives. Core c handles batch c//2,
sequence half c%2 (2048 query tokens + a 128-token look-backward halo whose
x rows ride along in the core's input shard; zeros at a true sequence start,
where the mask kills the backward keys anyway).

v2: fully fused single pass — no DRAM staging roundtrips (v1 spent ~60% of
its DMA on qrope/k2/vstage bounce buffers). Everything is bf16 (measured
gate error ~4.5e-3 against the 2e-2 budget):
  - x^T lands in SBUF via the DMA-engine transpose (InstDmaTransposeAnt,
    14ns/16x128 tile), eliminating v1's PE transposes + PSUM evictions.
  - QKV / out projections in bf16 (full PE rate at any free size).
  - q/k PSUM is evicted to SBUF bf16 (ACT), then RoPE runs on DVE at the
    2x 16-bit rate: 4 tensor_tensor ops per call over [128, 8*nw*128]
    views. Rotate partner of row r is r^64 via a host-side permutation of
    W_qkv columns; the sin sign is folded into the tables.
  - The banded causal mask is ADDED BY THE PE: one extra matmul per
    256-col half with an identity stationary and the additive mask as
    moving operand — no DVE mask traffic.
  - Softmax per head-pair: one ACT exp [128,512] PSUM->SBUF bf16, row
    sums via one DVE tensor_reduce [128,2,256]->[128,2] (f32), normalize
    fused with the bf16 store (tensor_scalar, 4x rate).
  - attn@v wants probs k-major: 4 PE transposes + one DVE copy per pair;
    the out-projection is fused per window (no full aT buffer).

Head-pair tasks are software-pipelined (stagger 2) so PE stays fed, and
chunk c's QKV overlaps chunk c-1's attention. GPSIMD/Pool is left idle on
purpose — it shares its SBUF port with DVE under an exclusive lock.
"""

import numpy as np
from contextlib import ExitStack
from ml_dtypes import bfloat16

import concourse.bacc as bacc
import concourse.tile as tile
import concourse.mybir as mybir
from concourse.bass_utils import run_bass_kernel_spmd
from concourse.masks import make_identity

# Problem shape (hardcoded per contract)
B, N, D = 4, 4096, 1024
H, DH, WS = 16, 64, 128
THETA = 10000.0
N3 = 3 * H * DH            # 3072
NCORES = 8
HALF = N // 2              # 2048 query tokens per core
NT = HALF + WS             # 2176 tokens incl halo window
SCALE = DH ** -0.5
NEG = -1.0e9
NCH = 9                    # chunks of 2 token-windows (last has 1)

F32 = mybir.dt.float32
BF16 = mybir.dt.bfloat16
ADD = mybir.AluOpType.add
MUL = mybir.AluOpType.mult
EXP = mybir.ActivationFunctionType.Exp
AXX = mybir.AxisListType.X


def _build(reps=1):
    nc = bacc.Bacc("TRN2", target_bir_lowering=False, debug=False,
                   enable_asserts=False, num_devices=NCORES)

    xs = nc.dram_tensor("xs", [NT, D], BF16, kind="ExternalInput").ap()
    wq = nc.dram_tensor("wq", [D, N3], BF16, kind="ExternalInput").ap()
    wo = nc.dram_tensor("wo", [D, D], BF16, kind="ExternalInput").ap()
    # 6 tables x [128 rows, 8 nch * 2 win * 128 cols] (tiled repeats)
    # 0:qcos 1:qsin 2:kcos_cur 3:ksin_cur 4:kcos_prev 5:ksin_prev
    ropes = nc.dram_tensor("ropes", [6, 128, 2048], BF16, kind="ExternalInput").ap()
    masks = nc.dram_tensor("masks", [2, 128, 256], BF16, kind="ExternalInput").ap()
    out = nc.dram_tensor("out", [HALF, D], F32, kind="ExternalOutput").ap()

    with tile.TileContext(nc) as tc:
        with ExitStack() as top:
            constp = top.enter_context(tc.tile_pool(name="const", bufs=1))
            identf = constp.tile([128, 128], F32, tag="idf")
            make_identity(nc, identf[:])
            identb = constp.tile([128, 128], BF16, tag="idb")
            nc.vector.tensor_copy(identb[:], identf[:])
            rp = constp.tile([128, 6, 2048], BF16, tag="ropes")
            nc.sync.dma_start(rp[:], ropes.rearrange("r p m -> p r m"))
            mk = constp.tile([128, 2, 256], BF16, tag="masks")
            nc.sync.dma_start(mk[:], masks.rearrange("r p m -> p r m"))

            rep_ctx = tc.For_i(0, reps, 1) if reps > 1 else ExitStack()
            top.enter_context(rep_ctx)

            wp = top.enter_context(tc.tile_pool(name="wqp", bufs=1))
            w_sb = wp.tile([128, 8, N3], BF16, tag="w")
            nc.sync.dma_start(w_sb[:], wq.rearrange("(c p) n -> p c n", p=128))
            wop = top.enter_context(tc.tile_pool(name="wop", bufs=1))
            wo_sb = wop.tile([128, 8, D], BF16, tag="wo")
            nc.sync.dma_start(wo_sb[:], wo.rearrange("(c p) n -> p c n", p=128))

            xTp = top.enter_context(tc.tile_pool(name="xT", bufs=2))
            qrawp = top.enter_context(tc.tile_pool(name="qraw", bufs=2))
            krawp = top.enter_context(tc.tile_pool(name="kraw", bufs=2))
            qtp = top.enter_context(tc.tile_pool(name="qt", bufs=2))
            kkp = top.enter_context(tc.tile_pool(name="kk", bufs=3))
            vp = top.enter_context(tc.tile_pool(name="v", bufs=3))
            tmpp = top.enter_context(tc.tile_pool(name="tmp", bufs=2))
            eep = top.enter_context(tc.tile_pool(name="ee", bufs=3))
            pfp = top.enter_context(tc.tile_pool(name="pf", bufs=3))
            ptp = top.enter_context(tc.tile_pool(name="pt", bufs=3))
            sump = top.enter_context(tc.tile_pool(name="sums", bufs=4))
            aTp = top.enter_context(tc.tile_pool(name="aTw", bufs=3))
            osbp = top.enter_context(tc.tile_pool(name="osb", bufs=2))

            mps = top.enter_context(tc.tile_pool(name="mps", bufs=2, space="PSUM"))
            sps = top.enter_context(tc.tile_pool(name="sps", bufs=3, space="PSUM"))
            ptqp = top.enter_context(tc.tile_pool(name="ptq", bufs=2, space="PSUM"))
            avp_ = top.enter_context(tc.tile_pool(name="avp", bufs=1, space="PSUM"))

            # cross-chunk state (python refs; pool bufs sized to live ranges)
            kk_tiles = {}
            v_tiles = {}
            qt_tiles = {}

            def tabv(i, nwv, r0, r1):
                # table slice [r1-r0, 8, nwv, 128]; content repeats per window
                return rp[r0:r1, i, :].rearrange(
                    "p (c w m) -> p c w m", w=2, m=128)[:, :, 0:nwv, :]

            def rope(dst_f, src_f, ci, si, nwv):
                """dst = src*cos + rot32(src)*sin_signed (6 DVE ops, bf16).

                dst_f/src_f(r0, r1) -> [r1-r0, 8, nwv, 128] APs. Contiguous
                per-head layout: rotate partner of row r is r^32 within each
                64-row head block, so the sin product needs 4 quarter-ops
                (only the OUTPUT of an op may be partition-shifted; the sin
                tile is indexed by SOURCE row, destination sign folded in
                host-side).
                """
                t1 = tmpp.tile([128, 8, 2, 128], BF16, tag="t1")
                t2 = tmpp.tile([128, 8, 2, 128], BF16, tag="t2")
                nc.vector.tensor_tensor(t1[:, :, 0:nwv, :], src_f(0, 128),
                                        tabv(ci, nwv, 0, 128), MUL)
                for g in (0, 1):
                    lo, hi = g * 64, g * 64 + 32
                    nc.vector.tensor_tensor(t2[lo:lo + 32, :, 0:nwv, :],
                                            src_f(hi, hi + 32),
                                            tabv(si, nwv, hi, hi + 32), MUL)
                    nc.vector.tensor_tensor(t2[hi:hi + 32, :, 0:nwv, :],
                                            src_f(lo, lo + 32),
                                            tabv(si, nwv, lo, lo + 32), MUL)
                nc.vector.tensor_tensor(dst_f(0, 128), t1[:, :, 0:nwv, :],
                                        t2[:, :, 0:nwv, :], ADD)

            def emit_qkv(c):
                nw = 2 if c < 8 else 1
                L = 128 * nw
                t0 = 2 * c
                xT = xTp.tile([128, 8, 256], BF16, tag="xT")
                nc.sync.dma_start_transpose(xT[:, :, 0:L],
                                            xs[t0 * 128: t0 * 128 + L, :])
                qs = 128 if c == 0 else 0

                qraw = qrawp.tile([128, 8, 256], BF16, tag="qr")
                kraw = krawp.tile([128, 8, 256], BF16, tag="kr")
                for sec, dst, s0 in ((0, qraw, qs), (1, kraw, 0)):
                    if L - s0 <= 0:
                        continue
                    for b2 in range(4):
                        mm = mps.tile([128, 512], F32, tag="mm")
                        for nch in (2 * b2, 2 * b2 + 1):
                            o = (nch % 2) * 256
                            for kc in range(8):
                                nc.tensor.matmul(
                                    mm[:, o + s0: o + L],
                                    w_sb[:, kc, 1024 * sec + nch * 128:
                                         1024 * sec + (nch + 1) * 128],
                                    xT[:, kc, s0:L],
                                    start=(kc == 0), stop=(kc == 7))
                        nc.scalar.copy(
                            dst[:, 2 * b2: 2 * b2 + 2, s0:L],
                            mm[:].rearrange("p (h m) -> p h m", h=2)[:, :, s0:L])

                # v in natural [token, 1024] layout
                vt = vp.tile([128, 2, D], BF16, tag="v")
                for mt in range(nw):
                    for nh in range(2):
                        vq = mps.tile([128, 512], F32, tag="mm")
                        for kc in range(8):
                            nc.tensor.matmul(
                                vq[:],
                                xT[:, kc, mt * 128:(mt + 1) * 128],
                                w_sb[:, kc, 2048 + nh * 512: 2048 + (nh + 1) * 512],
                                start=(kc == 0), stop=(kc == 7))
                        nc.scalar.copy(vt[:, mt, nh * 512:(nh + 1) * 512], vq[:])
                    v_tiles[t0 + mt] = (vt, mt)

                # --- RoPE (DVE, all-bf16 SBUF, 2x rate) ---
                if c + 1 <= NCH - 1 and (c + 1) not in kk_tiles:
                    kk_tiles[c + 1] = kkp.tile([128, 8, 2, 256], BF16, tag="kk", name="kk")
                if c not in kk_tiles:
                    kk_tiles[c] = kkp.tile([128, 8, 2, 256], BF16, tag="kk", name="kk")

                qt = qtp.tile([128, 8, 256], BF16, tag="qt")
                qt_tiles[c] = qt
                w0q = qs // 128

                def qdst(r0, r1):
                    return qt[r0:r1, :, :].rearrange(
                        "p c (w m) -> p c w m", m=128)[:, :, w0q:nw, :]

                def qsrc(r0, r1):
                    return qraw[r0:r1, :, :].rearrange(
                        "p c (w m) -> p c w m", m=128)[:, :, w0q:nw, :]

                rope(qdst, qsrc, 0, 1, nw - w0q)

                cs = 1 if c == 0 else 0     # halo window has no cur slot
                if nw - cs > 0:
                    def kcdst(r0, r1):
                        return kk_tiles[c][r0:r1].rearrange(
                            "p c s (h m) -> p c s h m", m=128)[:, :, cs:nw, 1, :]

                    def kcsrc(r0, r1):
                        return kraw[r0:r1, :, :].rearrange(
                            "p c (w m) -> p c w m", m=128)[:, :, cs:nw, :]

                    rope(kcdst, kcsrc, 2, 3, nw - cs)

                for mt in range(nw):
                    t = t0 + mt
                    if t > 15:              # last window has no next
                        continue
                    cw, sw = (t + 1) // 2, (t + 1) % 2

                    def kpdst(r0, r1, cw=cw, sw=sw):
                        return kk_tiles[cw][r0:r1].rearrange(
                            "p c s (h m) -> p c s h m", m=128)[:, :, sw:sw + 1, 0, :]

                    def kpsrc(r0, r1, mt=mt):
                        return kraw[r0:r1, :, :].rearrange(
                            "p c (w m) -> p c w m", m=128)[:, :, mt:mt + 1, :]

                    rope(kpdst, kpsrc, 4, 5, 1)

            def emit_scores(w, blk):
                qt = qt_tiles[w // 2]
                kk = kk_tiles[w // 2]
                slot = w % 2
                sp = sps.tile([128, 512], F32, tag="s")
                mvar = 0 if w == 1 else 1
                for sub in range(2):
                    o = sub * 256
                    po = sub * 64
                    nc.tensor.matmul(
                        sp[:, o:o + 256],
                        qt[po:po + 64, blk, slot * 128:(slot + 1) * 128],
                        kk[po:po + 64, blk, slot, :],
                        start=True, stop=False)
                    nc.tensor.matmul(sp[:, o:o + 256], identb[:],
                                     mk[:, mvar, :], start=False, stop=True)
                return sp

            def emit_rest(w, blk, sp, aTw):
                ee = eep.tile([128, 512], BF16, tag="ee")
                nc.scalar.activation(ee[:], sp[:], EXP)
                ss = sump.tile([128, 2], F32, tag="ss")
                nc.vector.tensor_reduce(
                    ss[:], ee[:].rearrange("p (h m) -> p h m", h=2),
                    axis=AXX, op=ADD)
                rr = sump.tile([128, 2], F32, tag="rr")
                nc.vector.reciprocal(rr[:], ss[:])
                pf = pfp.tile([128, 512], BF16, tag="pf")
                for hh in range(2):
                    nc.vector.tensor_scalar_mul(
                        pf[:, hh * 256:(hh + 1) * 256],
                        ee[:, hh * 256:(hh + 1) * 256], rr[:, hh:hh + 1])
                ptq = ptqp.tile([128, 512], BF16, tag="ptq")
                for j in range(4):
                    nc.tensor.transpose(ptq[:, j * 128:(j + 1) * 128],
                                        pf[:, j * 128:(j + 1) * 128], identb[:])
                pt = ptp.tile([128, 512], BF16, tag="pt")
                nc.vector.tensor_copy(pt[:], ptq[:])
                av = avp_.tile([128, 128], F32, tag="av")
                vprev, sprev = v_tiles[w - 1]
                vcur, scur = v_tiles[w]
                for sub in range(2):
                    d0 = blk * 128 + sub * 64
                    nc.tensor.matmul(av[sub * 64:(sub + 1) * 64, :],
                                     vprev[:, sprev, d0:d0 + 64],
                                     pt[:, sub * 256: sub * 256 + 128],
                                     start=True, stop=False)
                    nc.tensor.matmul(av[sub * 64:(sub + 1) * 64, :],
                                     vcur[:, scur, d0:d0 + 64],
                                     pt[:, sub * 256 + 128: sub * 256 + 256],
                                     start=False, stop=True)
                nc.scalar.copy(aTw[:, blk, :], av[:])

            def emit_outproj(w, aTw):
                osb = osbp.tile([128, D], F32, tag="o")
                for nh in range(2):
                    op_ = mps.tile([128, 512], F32, tag="mm")
                    for kc in range(8):
                        nc.tensor.matmul(op_[:], aTw[:, kc, :],
                                         wo_sb[:, kc, nh * 512:(nh + 1) * 512],
                                         start=(kc == 0), stop=(kc == 7))
                    nc.scalar.copy(osb[:, nh * 512:(nh + 1) * 512], op_[:])
                nc.sync.dma_start(out[(w - 1) * 128: w * 128, :], osb[:])

            # ---- software-pipelined main loop ----
            S = 2  # head-pair stagger depth
            pend = []
            aTw_tiles = {}

            def drain_one():
                w, blk, sp, aTw = pend.pop(0)
                emit_rest(w, blk, sp, aTw)
                if blk == 7:
                    emit_outproj(w, aTw)
                    del aTw_tiles[w]

            def attn_windows(ws):
                for w in ws:
                    aTw_tiles[w] = aTp.tile([128, 8, 128], BF16, tag="aTw", name="aTw")
                for w in ws:
                    for blk in range(8):
                        sp = emit_scores(w, blk)
                        pend.append((w, blk, sp, aTw_tiles[w]))
                        while len(pend) > S:
                            drain_one()

            for c in range(NCH + 1):
                if c >= 1:
                    lo = 2 * (c - 1)
                    ws = [t for t in (lo, lo + 1) if 1 <= t <= 16]
                    attn_windows(ws)
                if c < NCH:
                    emit_qkv(c)
            while pend:
                drain_one()

    nc.compile()
    return nc


_NC = {}


def _get_nc(reps=1):
    if reps not in _NC:
        _NC[reps] = _build(reps)
    return _NC[reps]


# contiguous per-head layout: each 128-row block is [hA d0-63 | hB d0-63];
# rotate partner of row r is r^32 within each 64-row head block.
_r = np.arange(128)


def _host_inputs(x, W_qkv, W_out):
    Wb = np.ascontiguousarray(W_qkv, np.float32).astype(bfloat16)
    Wob = np.ascontiguousarray(W_out, np.float32).astype(bfloat16)

    invf = THETA ** (-(np.arange(0, 64, 2) / 64.0))          # [32]
    rows_f = invf[_r % 32]                                   # [128] freq per row
    # sin tiles are indexed by SOURCE row of the rotate (partner r^32);
    # the destination sign is +1 when the source is the hi half of its
    # 64-row head block (rot(t) = [-t_hi, t_lo]).
    rows_s = np.where((_r % 64) < 32, 1.0, -1.0)
    mcol = np.arange(128)
    angC = rows_f[:, None] * (128 + mcol)[None, :]
    angP = rows_f[:, None] * mcol[None, :]
    tabs = np.stack([
        SCALE * np.cos(angC),
        SCALE * (rows_s[:, None] * np.sin(angC)),
        np.cos(angC),
        rows_s[:, None] * np.sin(angC),
        np.cos(angP),
        rows_s[:, None] * np.sin(angP),
    ])                                                       # [6,128,128]
    ropes = np.tile(tabs, (1, 1, 16)).astype(bfloat16)       # [6,128,2048]

    i = np.arange(128)[:, None]
    jj = np.arange(256)[None, :]
    band = (jj >= i) & (jj <= i + 128)
    maskB = np.where(band, 0.0, NEG).astype(bfloat16)
    maskA0 = np.where(band & (jj >= 128), 0.0, NEG).astype(bfloat16)

    in_maps = []
    for c in range(NCORES):
        bi, hi = c // 2, c % 2
        xsh = np.empty((NT, D), np.float32)
        if hi == 0:
            xsh[:WS] = 0.0
            xsh[WS:] = x[bi, 0:HALF]
            mA = maskA0
        else:
            xsh[:] = x[bi, HALF - WS: N]
            mA = maskB
        in_maps.append({
            "xs": xsh.astype(bfloat16),
            "wq": Wb,
            "wo": Wob,
            "ropes": ropes,
            "masks": np.stack([mA, maskB]),
        })
    return in_maps


def kernel(x, W_qkv, W_out):
    x = np.asarray(x, np.float32)
    nc = _get_nc()
    in_maps = _host_inputs(x, W_qkv, W_out)
    res = run_bass_kernel_spmd(nc, in_maps, list(range(NCORES)))
    outf = np.empty((B, N, D), np.float32)
    for c in range(NCORES):
        bi, hi = c // 2, c % 2
        outf[bi, hi * HALF:(hi + 1) * HALF] = res.results[c]["out"]
    return outf


# revision 23
# speedup vs baseline: 1.8846x; 1.1579x over previous
"""LocalMHA (windowed attention, window=128, look_backward=1, RoPE) on 8 TRN2 cores.

Sharding: sequence-parallel, no collectives. Core c handles batch c//2,
sequence half c%2 (2048 query tokens + a 128-token look-backward halo whose
x rows ride along in the core's input shard; zeros at a true sequence start,
where the mask kills the backward keys anyway).

v2: fully fused single pass — no DRAM staging roundtrips (v1 spent ~60% of
its DMA on qrope/k2/vstage bounce buffers). Everything is bf16 (measured
gate error ~4.5e-3 against the 2e-2 budget):
  - x^T lands in SBUF via the DMA-engine transpose (InstDmaTransposeAnt,
    14ns/16x128 tile), eliminating v1's PE transposes + PSUM evictions.
  - QKV / out projections in bf16 (full PE rate at any free size).
  - q/k PSUM is evicted to SBUF bf16 (ACT), then RoPE runs on DVE at the
    2x 16-bit rate over [128, 8, nw, 128] views. Contiguous per-head
    layout: rotate partner of row r is r^32, so the sin product is 4
    quarter-ops (6 DVE ops per call); sin sign is folded into the tables.
  - The banded causal mask is ADDED BY THE PE: one matmul per pair with an
    identity stationary and the [mask|mask] tile as moving operand
    accumulates mask[i,j] into the scores PSUM — no DVE mask traffic.
  - Softmax per head-pair: 2 ACT exps [128,256] PSUM->SBUF bf16 with
    fused row-sum accumulators, then DVE reciprocal + two 4x-rate
    tensor_scalar normalize-and-store ops.
  - attn@v wants probs k-major: 4 PE transposes + one DVE copy per pair;
    the out-projection is fused per window (no full aT buffer).

Head-pair tasks are software-pipelined (stagger 3) so PE stays fed; chunk
c's QKV overlaps chunk c-1's attention, and x^T DMA-transposes are
prefetched one chunk ahead. GPSIMD/Pool is left idle on purpose — it
shares its SBUF port with DVE under an exclusive lock.
"""

import numpy as np
from contextlib import ExitStack
from ml_dtypes import bfloat16

import concourse.bacc as bacc
import concourse.tile as tile
import concourse.mybir as mybir
from concourse.bass_utils import run_bass_kernel_spmd
from concourse.masks import make_identity

# Problem shape (hardcoded per contract)
B, N, D = 4, 4096, 1024
H, DH, WS = 16, 64, 128
THETA = 10000.0
N3 = 3 * H * DH            # 3072
NCORES = 8
HALF = N // 2              # 2048 query tokens per core
NT = HALF + WS             # 2176 tokens incl halo window
SCALE = DH ** -0.5
NEG = -1.0e9
CW = 4                     # token-windows per chunk
NCH = 5                    # chunks (last has 1 window)

F32 = mybir.dt.float32
BF16 = mybir.dt.bfloat16
ADD = mybir.AluOpType.add
MUL = mybir.AluOpType.mult
EXP = mybir.ActivationFunctionType.Exp


def _build(reps=1):
    nc = bacc.Bacc("TRN2", target_bir_lowering=False, debug=False,
                   enable_asserts=False, num_devices=NCORES)

    xs = nc.dram_tensor("xs", [NT, D], BF16, kind="ExternalInput").ap()
    wq = nc.dram_tensor("wq", [D, N3], BF16, kind="ExternalInput").ap()
    wo = nc.dram_tensor("wo", [D, D], BF16, kind="ExternalInput").ap()
    # 6 tables x [128 rows, 8 nch * CW win * 128 cols] (tiled repeats)
    # 0:qcos 1:qsin 2:kcos_cur 3:ksin_cur 4:kcos_prev 5:ksin_prev
    ropes = nc.dram_tensor("ropes", [6, 128, 128], BF16,
                           kind="ExternalInput").ap()
    masks = nc.dram_tensor("masks", [2, 128, 512], BF16, kind="ExternalInput").ap()
    out = nc.dram_tensor("out", [HALF, D], F32, kind="ExternalOutput").ap()

    with tile.TileContext(nc) as tc:
        with ExitStack() as top:
            constp = top.enter_context(tc.tile_pool(name="const", bufs=1))
            identf = constp.tile([128, 128], F32, tag="idf")
            make_identity(nc, identf[:])
            identb = constp.tile([128, 128], BF16, tag="idb")
            nc.vector.tensor_copy(identb[:], identf[:])
            rp = constp.tile([128, 6, 1, 1, 128], BF16, tag="ropes")
            nc.sync.dma_start(rp[:, :, 0, 0, :], ropes.rearrange("r p m -> p r m"))
            mk = constp.tile([128, 2, 512], BF16, tag="masks")
            nc.sync.dma_start(mk[:], masks.rearrange("r p m -> p r m"))

            rep_ctx = tc.For_i(0, reps, 1) if reps > 1 else ExitStack()
            top.enter_context(rep_ctx)

            wp = top.enter_context(tc.tile_pool(name="wqp", bufs=1))
            w_sb = wp.tile([128, 8, N3], BF16, tag="w")
            nc.sync.dma_start(w_sb[:], wq.rearrange("(c p) n -> p c n", p=128))
            wop = top.enter_context(tc.tile_pool(name="wop", bufs=1))
            wo_sb = wop.tile([128, 8, D], BF16, tag="wo")
            nc.sync.dma_start(wo_sb[:], wo.rearrange("(c p) n -> p c n", p=128))

            xTp = top.enter_context(tc.tile_pool(name="xT", bufs=2))
            qrawp = top.enter_context(tc.tile_pool(name="qraw", bufs=1))
            krawp = top.enter_context(tc.tile_pool(name="kraw", bufs=1))
            qtp = top.enter_context(tc.tile_pool(name="qt", bufs=2))
            kkp = top.enter_context(tc.tile_pool(name="kk", bufs=2))
            vp = top.enter_context(tc.tile_pool(name="v", bufs=2))
            tmpp = top.enter_context(tc.tile_pool(name="tmp", bufs=1))
            eep = top.enter_context(tc.tile_pool(name="ee", bufs=4))
            pfp = top.enter_context(tc.tile_pool(name="pf", bufs=4))
            ptp = top.enter_context(tc.tile_pool(name="pt", bufs=4))
            sump = top.enter_context(tc.tile_pool(name="sums", bufs=6))
            aTp = top.enter_context(tc.tile_pool(name="aTw", bufs=3))
            osbp = top.enter_context(tc.tile_pool(name="osb", bufs=2))

            mps = top.enter_context(tc.tile_pool(name="mps", bufs=2, space="PSUM"))
            sps = top.enter_context(tc.tile_pool(name="sps", bufs=4, space="PSUM"))
            ptqp = top.enter_context(tc.tile_pool(name="ptq", bufs=1, space="PSUM"))
            avp_ = top.enter_context(tc.tile_pool(name="avp", bufs=1, space="PSUM"))

            # cross-chunk state (python refs; pool bufs sized to live ranges)
            kk_tiles = {}
            v_tiles = {}
            qt_tiles = {}
            xT_tiles = {}

            def nwof(c):
                return CW if c < NCH - 1 else 1

            def prefetch_xT(c):
                nw = nwof(c)
                L = 128 * nw
                t0 = CW * c
                xT = xTp.tile([128, 8, CW * 128], BF16, tag="xT", name="xT")
                nc.sync.dma_start_transpose(xT[:, :, 0:L],
                                            xs[t0 * 128: t0 * 128 + L, :])
                xT_tiles[c] = xT

            def tabv(i, nwv, r0, r1):
                # stride-0 broadcast over (nch, window): table is one window
                return rp[r0:r1, i].broadcast_to([r1 - r0, 8, nwv, 128])

            def rope(dst_f, src_f, ci, si, nwv):
                """dst = src*cos + rot32(src)*sin_signed (6 DVE ops, bf16).

                dst_f/src_f(r0, r1) -> [r1-r0, 8, nwv, 128] APs. Contiguous
                per-head layout: rotate partner of row r is r^32 within each
                64-row head block, so the sin product needs 4 quarter-ops
                (only the OUTPUT of an op may be partition-shifted; the sin
                tile is indexed by SOURCE row, destination sign folded in
                host-side).
                """
                t1 = tmpp.tile([128, 8, CW, 128], BF16, tag="t1")
                t2 = tmpp.tile([128, 8, CW, 128], BF16, tag="t2")
                nc.vector.tensor_tensor(t1[:, :, 0:nwv, :], src_f(0, 128),
                                        tabv(ci, nwv, 0, 128), MUL)
                for g in (0, 1):
                    lo, hi = g * 64, g * 64 + 32
                    nc.vector.tensor_tensor(t2[lo:lo + 32, :, 0:nwv, :],
                                            src_f(hi, hi + 32),
                                            tabv(si, nwv, hi, hi + 32), MUL)
                    nc.vector.tensor_tensor(t2[hi:hi + 32, :, 0:nwv, :],
                                            src_f(lo, lo + 32),
                                            tabv(si, nwv, lo, lo + 32), MUL)
                nc.vector.tensor_tensor(dst_f(0, 128), t1[:, :, 0:nwv, :],
                                        t2[:, :, 0:nwv, :], ADD)

            def emit_kproj(c):
                nw = nwof(c)
                L = 128 * nw
                t0 = CW * c
                xT = xT_tiles[c]

                if c + 1 <= NCH - 1 and (c + 1) not in kk_tiles:
                    kk_tiles[c + 1] = kkp.tile([128, 8, CW, 256], BF16,
                                               tag="kk", name="kk")
                if c not in kk_tiles:
                    kk_tiles[c] = kkp.tile([128, 8, CW, 256], BF16,
                                           tag="kk", name="kk")

                # K first: its ropes gate the next window group's scores, so
                # they run on DVE while PE chews the previous chunk's
                # attention matmuls.
                kraw = krawp.tile([128, 8, CW * 128], BF16, tag="kr")
                for nch in range(8):
                    mm = mps.tile([128, 512], F32, tag="mm")
                    for kc in range(8):
                        nc.tensor.matmul(
                            mm[:, 0:L],
                            w_sb[:, kc, 1024 + nch * 128: 1024 + (nch + 1) * 128],
                            xT[:, kc, 0:L],
                            start=(kc == 0), stop=(kc == 7))
                    nc.scalar.copy(kraw[:, nch, 0:L], mm[:, 0:L])

                # Per-window rope pieces, deferred: they interleave with the
                # previous chunk's attention pairs so no long serial rope
                # block ever stalls the fine-grained DVE stream.
                cs = 1 if c == 0 else 0     # halo window has no cur slot
                for mt in range(cs, nw):
                    def kcur_piece(mt=mt, kraw=kraw, c=c):
                        def kcdst(r0, r1):
                            return kk_tiles[c][r0:r1].rearrange(
                                "p c s (h m) -> p c s h m",
                                m=128)[:, :, mt:mt + 1, 1, :]

                        def kcsrc(r0, r1):
                            return kraw[r0:r1, :, :].rearrange(
                                "p c (w m) -> p c w m", m=128)[:, :, mt:mt + 1, :]

                        rope(kcdst, kcsrc, 2, 3, 1)
                    rope_pieces.append(kcur_piece)

                # kprv: token-window t feeds query window t+1's prv half
                for mt in range(nw):
                    t = t0 + mt
                    if t > 15:
                        continue
                    cw_, sw = (t + 1) // CW, (t + 1) % CW

                    def kprv_piece(mt=mt, kraw=kraw, cw_=cw_, sw=sw):
                        def kpdst(r0, r1):
                            return kk_tiles[cw_][r0:r1].rearrange(
                                "p c s (h m) -> p c s h m",
                                m=128)[:, :, sw:sw + 1, 0, :]

                        def kpsrc(r0, r1):
                            return kraw[r0:r1, :, :].rearrange(
                                "p c (w m) -> p c w m", m=128)[:, :, mt:mt + 1, :]

                        rope(kpdst, kpsrc, 4, 5, 1)
                    rope_pieces.append(kprv_piece)

            def emit_qproj(c):
                nw = nwof(c)
                L = 128 * nw
                xT = xT_tiles[c]
                qs = 128 if c == 0 else 0
                qraw = qrawp.tile([128, 8, CW * 128], BF16, tag="qr")
                for nch in range(8):
                    mm = mps.tile([128, 512], F32, tag="mm")
                    for kc in range(8):
                        nc.tensor.matmul(
                            mm[:, qs:L],
                            w_sb[:, kc, nch * 128:(nch + 1) * 128],
                            xT[:, kc, qs:L],
                            start=(kc == 0), stop=(kc == 7))
                    nc.scalar.copy(qraw[:, nch, qs:L], mm[:, qs:L])

                qt = qtp.tile([128, 8, CW * 128], BF16, tag="qt")
                qt_tiles[c] = qt
                w0q = qs // 128

                for mt in range(w0q, nw):
                    def q_piece(mt=mt, qraw=qraw, qt=qt):
                        def qdst(r0, r1):
                            return qt[r0:r1, :, :].rearrange(
                                "p c (w m) -> p c w m", m=128)[:, :, mt:mt + 1, :]

                        def qsrc(r0, r1):
                            return qraw[r0:r1, :, :].rearrange(
                                "p c (w m) -> p c w m", m=128)[:, :, mt:mt + 1, :]

                        rope(qdst, qsrc, 0, 1, 1)
                    rope_pieces.append(q_piece)

            def emit_vproj(c):
                nw = nwof(c)
                t0 = CW * c
                xT = xT_tiles.pop(c)
                # V natural [token, 1024] layout; evictions on DVE to keep
                # ACT free for the attention exps.
                vt = vp.tile([128, CW, D], BF16, tag="v")
                for mt in range(nw):
                    for nh in range(2):
                        vq = mps.tile([128, 512], F32, tag="mm")
                        for kc in range(8):
                            nc.tensor.matmul(
                                vq[:],
                                xT[:, kc, mt * 128:(mt + 1) * 128],
                                w_sb[:, kc, 2048 + nh * 512: 2048 + (nh + 1) * 512],
                                start=(kc == 0), stop=(kc == 7))
                        nc.scalar.copy(vt[:, mt, nh * 512:(nh + 1) * 512],
                                       vq[:])
                    v_tiles[t0 + mt] = (vt, mt)

            def emit_scores(w, blk):
                qt = qt_tiles[w // CW]
                kk = kk_tiles[w // CW]
                slot = w % CW
                sp = sps.tile([128, 512], F32, tag="s")
                mvar = 0 if w == 1 else 1
                for sub in range(2):
                    o = sub * 256
                    po = sub * 64
                    nc.tensor.matmul(
                        sp[:, o:o + 256],
                        qt[po:po + 64, blk, slot * 128:(slot + 1) * 128],
                        kk[po:po + 64, blk, slot, :],
                        start=True, stop=False)
                    nc.tensor.matmul(sp[:, o:o + 256], identb[:],
                                     mk[:, mvar, o:o + 256],
                                     start=False, stop=True)
                return sp

            def emit_rest(w, blk, sp, aTw):
                # Row sums alternate between ACT (fused exp accumulators) and
                # DVE (tensor_reduce) to balance the two engines.
                ee = eep.tile([128, 512], BF16, tag="ee")
                ss = sump.tile([128, 2], F32, tag="ss")
                if blk % 2 == 0:
                    for hh in range(2):
                        nc.scalar.activation(ee[:, hh * 256:(hh + 1) * 256],
                                             sp[:, hh * 256:(hh + 1) * 256], EXP,
                                             accum_out=ss[:, hh:hh + 1])
                else:
                    nc.scalar.activation(ee[:], sp[:], EXP)
                    nc.vector.tensor_reduce(
                        ss[:], ee[:].rearrange("p (h m) -> p h m", h=2),
                        axis=mybir.AxisListType.X, op=ADD)
                rr = sump.tile([128, 2], F32, tag="rr")
                nc.vector.reciprocal(rr[:], ss[:])
                pf = pfp.tile([128, 512], BF16, tag="pf")
                for hh in range(2):
                    nc.vector.tensor_scalar_mul(
                        pf[:, hh * 256:(hh + 1) * 256],
                        ee[:, hh * 256:(hh + 1) * 256], rr[:, hh:hh + 1])
                ptq = ptqp.tile([128, 512], BF16, tag="ptq")
                for j in range(4):
                    nc.tensor.transpose(ptq[:, j * 128:(j + 1) * 128],
                                        pf[:, j * 128:(j + 1) * 128], identb[:])
                pt = ptp.tile([128, 512], BF16, tag="pt")
                nc.vector.tensor_copy(pt[:], ptq[:])
                av = avp_.tile([128, 128], F32, tag="av")
                vprev, sprev = v_tiles[w - 1]
                vcur, scur = v_tiles[w]
                for sub in range(2):
                    d0 = blk * 128 + sub * 64
                    nc.tensor.matmul(av[sub * 64:(sub + 1) * 64, :],
                                     vprev[:, sprev, d0:d0 + 64],
                                     pt[:, sub * 256: sub * 256 + 128],
                                     start=True, stop=False)
                    nc.tensor.matmul(av[sub * 64:(sub + 1) * 64, :],
                                     vcur[:, scur, d0:d0 + 64],
                                     pt[:, sub * 256 + 128: sub * 256 + 256],
                                     start=False, stop=True)
                nc.scalar.copy(aTw[:, blk, :], av[:])

            def emit_outproj(w, aTw):
                osb = osbp.tile([128, D], F32, tag="o")
                for nh in range(2):
                    op_ = mps.tile([128, 512], F32, tag="mm")
                    for kc in range(8):
                        nc.tensor.matmul(op_[:], aTw[:, kc, :],
                                         wo_sb[:, kc, nh * 512:(nh + 1) * 512],
                                         start=(kc == 0), stop=(kc == 7))
                    nc.scalar.copy(osb[:, nh * 512:(nh + 1) * 512], op_[:])
                nc.sync.dma_start(out[(w - 1) * 128: w * 128, :], osb[:])

            # ---- software-pipelined main loop ----
            S = 3  # head-pair stagger depth
            pend = []
            aTw_tiles = {}
            rope_pieces = []

            drain_n = [0]

            def drain_one():
                w, blk, sp, aTw = pend.pop(0)
                emit_rest(w, blk, sp, aTw)
                drain_n[0] += 1
                if rope_pieces and drain_n[0] % 3 == 0:
                    rope_pieces.pop(0)()
                if blk == 7:
                    emit_outproj(w, aTw)
                    del aTw_tiles[w]

            def attn_windows(ws):
                for w in ws:
                    aTw_tiles[w] = aTp.tile([128, 8, 128], BF16, tag="aTw",
                                            name="aTw")
                    for blk in range(8):
                        sp = emit_scores(w, blk)
                        pend.append((w, blk, sp, aTw_tiles[w]))
                        while len(pend) > S:
                            drain_one()

            prefetch_xT(0)
            for c in range(NCH + 1):
                if c + 1 <= NCH - 1:
                    prefetch_xT(c + 1)
                if c < NCH:
                    emit_kproj(c)
                    emit_qproj(c)
                if c >= 1:
                    lo = CW * (c - 1)
                    ws = [t for t in range(lo, lo + CW) if 1 <= t <= 16]
                    attn_windows(ws)
                if c < NCH:
                    emit_vproj(c)
                while rope_pieces:
                    rope_pieces.pop(0)()
            while pend:
                drain_one()

    nc.compile()
    return nc


_NC = {}


def _get_nc(reps=1):
    if reps not in _NC:
        _NC[reps] = _build(reps)
    return _NC[reps]


# contiguous per-head layout: each 128-row block is [hA d0-63 | hB d0-63];
# rotate partner of row r is r^32 within each 64-row head block.
_r = np.arange(128)


def _host_inputs(x, W_qkv, W_out):
    Wb = np.ascontiguousarray(W_qkv, np.float32).astype(bfloat16)
    Wob = np.ascontiguousarray(W_out, np.float32).astype(bfloat16)

    invf = THETA ** (-(np.arange(0, 64, 2) / 64.0))          # [32]
    rows_f = invf[_r % 32]                                   # [128] freq per row
    # sin tiles are indexed by SOURCE row of the rotate (partner r^32);
    # the destination sign is +1 when the source is the lo half of its
    # 64-row head block (rot(t) = [-t_hi, t_lo]).
    rows_s = np.where((_r % 64) < 32, 1.0, -1.0)
    mcol = np.arange(128)
    angC = rows_f[:, None] * (128 + mcol)[None, :]
    angP = rows_f[:, None] * mcol[None, :]
    tabs = np.stack([
        SCALE * np.cos(angC),
        SCALE * (rows_s[:, None] * np.sin(angC)),
        np.cos(angC),
        rows_s[:, None] * np.sin(angC),
        np.cos(angP),
        rows_s[:, None] * np.sin(angP),
    ])                                                       # [6,128,128]
    ropes = tabs.astype(bfloat16)                            # [6,128,128]

    i = np.arange(128)[:, None]
    jj = np.arange(256)[None, :]
    band = (jj >= i) & (jj <= i + 128)
    maskB = np.where(band, 0.0, NEG).astype(np.float32)
    maskA0 = np.where(band & (jj >= 128), 0.0, NEG).astype(np.float32)
    mB2 = np.concatenate([maskB, maskB], axis=1).astype(bfloat16)
    mA2 = np.concatenate([maskA0, maskA0], axis=1).astype(bfloat16)

    in_maps = []
    for c in range(NCORES):
        bi, hi = c // 2, c % 2
        xsh = np.empty((NT, D), np.float32)
        if hi == 0:
            xsh[:WS] = 0.0
            xsh[WS:] = x[bi, 0:HALF]
            mA = mA2
        else:
            xsh[:] = x[bi, HALF - WS: N]
            mA = mB2
        in_maps.append({
            "xs": xsh.astype(bfloat16),
            "wq": Wb,
            "wo": Wob,
            "ropes": ropes,
            "masks": np.stack([mA, mB2]),
        })
    return in_maps


def kernel(x, W_qkv, W_out):
    x = np.asarray(x, np.float32)
    nc = _get_nc()
    in_maps = _host_inputs(x, W_qkv, W_out)
    res = run_bass_kernel_spmd(nc, in_maps, list(range(NCORES)))
    outf = np.empty((B, N, D), np.float32)
    for c in range(NCORES):
        bi, hi = c // 2, c % 2
        outf[bi, hi * HALF:(hi + 1) * HALF] = res.results[c]["out"]
    return outf
